# revision 2
# baseline (speedup 1.0000x reference)
"""GroupConnected segment-reduce kernel for 8x Trainium2 NeuronCores.

Computes out[b, g] = sum_k x[b, idx[g, k]] * w[g, k] for
B=8192, F=8192, G=2048, K=8 (f32), sharding batch B across 8 cores.

Per-core pipeline:
  Phase T: transpose x_local [Bc, F] -> DRAM scratch xT [F, Bc]
           (PE transpose 128x128 blocks, DVE PSUM->SBUF copies,
            column-panel buffering so every DMA moves large tiles).
  Phase G: for each g-tile (128 groups) and k-slot: indirect-DMA gather
           the 128 rows xT[idx[g,k], :] (4KB descriptors), multiply by
           diag(w[:, k]) on the PE accumulating over k in PSUM,
           -> outT [G, Bc].  Host transposes outT back to [Bc, G].
"""

import numpy as np

import concourse.bacc as bacc
import concourse.tile as tile
from concourse import bass, mybir
from concourse.bass_utils import run_bass_kernel_spmd
from concourse.masks import make_identity

P = 128

# Problem shape (hardcoded per contract).
B, F, G, K = 8192, 8192, 2048, 8
N_CORES = 8
BC = B // N_CORES  # 1024 batch rows per core

_compiled = {}


def build_program(Bc=BC, Fdim=F, Gdim=G, Kdim=K, panel_f=512, trace_label=None):
    """Build the SPMD Bass program (same program on every core)."""
    assert Bc % P == 0 and Fdim % panel_f == 0 and panel_f % P == 0
    assert Gdim % P == 0
    n_gt = Gdim // P          # g-tiles
    n_bb = Bc // P            # b-blocks of 128
    n_pan = Fdim // panel_f   # column panels
    n_sub = panel_f // P      # 128-wide f sub-chunks per panel
    n_half = max(1, Bc // 512)  # 512-wide rhs slices per matmul group
    half = min(Bc, 512)

    nc = bacc.Bacc("TRN2", target_bir_lowering=False, debug=False)

    x_in = nc.dram_tensor("x_local", [Bc, Fdim], mybir.dt.float32,
                          kind="ExternalInput").ap()
    w_in = nc.dram_tensor("w_in", [Gdim, Kdim], mybir.dt.float32,
                          kind="ExternalInput").ap()
    gidx_in = nc.dram_tensor("gidx", [P, n_gt * Kdim], mybir.dt.int32,
                             kind="ExternalInput").ap()
    out_t = nc.dram_tensor("outT", [Gdim, Bc], mybir.dt.float32,
                           kind="ExternalOutput").ap()
    xT = nc.dram_tensor("xT", [Fdim, Bc], mybir.dt.float32).ap()

    with tile.TileContext(nc) as tc:
        with tc.tile_pool(name="const", bufs=1) as cpool:
            ident = cpool.tile([P, P], dtype=mybir.dt.float32)
            make_identity(nc, ident[:])

            # ---------------- Phase T: transpose x_local -> xT ----------
            with tc.tile_pool(name="tload", bufs=3) as lpool, \
                 tc.tile_pool(name="tpsum", bufs=4, space="PSUM") as tpsum, \
                 tc.tile_pool(name="tpanel", bufs=2) as ppool:
                for p in range(n_pan):
                    panels = [ppool.tile([P, Bc], dtype=mybir.dt.float32,
                                         name=f"xtp{c}", tag=f"xtp{c}")
                              for c in range(n_sub)]
                    for j in range(n_bb):
                        xb = lpool.tile([P, panel_f], dtype=mybir.dt.float32)
                        nc.sync.dma_start(
                            out=xb[:],
                            in_=x_in[j * P:(j + 1) * P,
                                     p * panel_f:(p + 1) * panel_f])
                        for c in range(n_sub):
                            ps = tpsum.tile([P, P], dtype=mybir.dt.float32,
                                            space="PSUM")
                            nc.tensor.transpose(
                                out=ps[:], in_=xb[:, c * P:(c + 1) * P],
                                identity=ident[:])
                            nc.vector.tensor_copy(
                                out=panels[c][:, j * P:(j + 1) * P], in_=ps[:])
                    for c in range(n_sub):
                        row0 = p * panel_f + c * P
                        nc.sync.dma_start(out=xT[row0:row0 + P, :],
                                          in_=panels[c][:])

            # xT fully written before any gather reads it.
            tc.strict_bb_all_engine_barrier()

            # ---------------- Phase G: gather + diag-matmul reduce ------
            with tc.tile_pool(name="gconst", bufs=1) as gc, \
                 tc.tile_pool(name="gath", bufs=4) as apool, \
                 tc.tile_pool(name="gpsum", bufs=2, space="PSUM") as gpsum, \
                 tc.tile_pool(name="gout", bufs=2) as opool:
                w_sb = gc.tile([P, n_gt * Kdim], dtype=mybir.dt.float32)
                for t in range(n_gt):
                    nc.sync.dma_start(
                        out=w_sb[:, t * Kdim:(t + 1) * Kdim],
                        in_=w_in[t * P:(t + 1) * P, :])
                gidx_sb = gc.tile([P, n_gt * Kdim], dtype=mybir.dt.int32)
                nc.sync.dma_start(out=gidx_sb[:], in_=gidx_in[:])

                # diag(w) blocks, one [128,128] per (t, k), built on DVE.
                diag = gc.tile([P, n_gt * Kdim * P], dtype=mybir.dt.float32)
                for j in range(n_gt * Kdim):
                    nc.vector.tensor_scalar(
                        out=diag[:, j * P:(j + 1) * P], in0=ident[:],
                        scalar1=w_sb[:, j:j + 1], scalar2=None,
                        op0=mybir.AluOpType.mult)

                for t in range(n_gt):
                    psum_t = gpsum.tile([P, Bc], dtype=mybir.dt.float32,
                                        space="PSUM")
                    for k in range(Kdim):
                        j = t * Kdim + k
                        a = apool.tile([P, Bc], dtype=mybir.dt.float32)
                        nc.gpsimd.indirect_dma_start(
                            out=a[:], out_offset=None, in_=xT[:, :],
                            in_offset=bass.IndirectOffsetOnAxis(
                                ap=gidx_sb[:, j:j + 1], axis=0))
                        for h in range(n_half):
                            nc.tensor.matmul(
                                out=psum_t[:, h * half:(h + 1) * half],
                                lhsT=diag[:, j * P:(j + 1) * P],
                                rhs=a[:, h * half:(h + 1) * half],
                                start=(k == 0), stop=(k == Kdim - 1))
                    osb = opool.tile([P, Bc], dtype=mybir.dt.float32)
                    nc.vector.tensor_copy(out=osb[:], in_=psum_t[:])
                    nc.sync.dma_start(out=out_t[t * P:(t + 1) * P, :],
                                      in_=osb[:])

    nc.compile()
    return nc


def _get_program():
    if "full" not in _compiled:
        _compiled["full"] = build_program()
    return _compiled["full"]


def kernel(x, group_idx, w, _trace=False):
    x = np.ascontiguousarray(np.asarray(x), dtype=np.float32)
    w_np = np.ascontiguousarray(np.asarray(w), dtype=np.float32)
    gi = np.asarray(group_idx).astype(np.int64)

    # gather-index table: column t*K+k holds idx[t*128:(t+1)*128, k] (int32)
    n_gt = G // P
    tbl = np.empty((P, n_gt * K), dtype=np.int32)
    for t in range(n_gt):
        tbl[:, t * K:(t + 1) * K] = gi[t * P:(t + 1) * P, :].astype(np.int32)

    nc = _get_program()
    in_maps = [
        {"x_local": x[i * BC:(i + 1) * BC, :], "w_in": w_np, "gidx": tbl}
        for i in range(N_CORES)
    ]
    res = run_bass_kernel_spmd(nc, in_maps, list(range(N_CORES)),
                               trace=_trace)
    out = np.empty((B, G), dtype=np.float32)
    for i in range(N_CORES):
        out[i * BC:(i + 1) * BC, :] = res.results[i]["outT"].T
    if _trace:
        return out, res
    return out
